# revision 4
# baseline (speedup 1.0000x reference)
"""Distance-based attention (nn_Attention_67989332296336) on 8 TRN2 NeuronCores.

Math per batch element b (S=1024, E=H=A=256):
    d2[t,j]  = |x_t|^2 + |x_j|^2 - 2 x_t.x_j
    dist     = sqrt(max(d2,0)+eps)
    scores   = w_sim*dist + b_sim
    A        = softmax_j(scores)
    G        = A @ h
    Z        = tanh([G, h] @ W_g^T + b_g)

Sharding: batch dim B=32 split over 8 cores (4 per core), weights replicated.

Per-core kernel strategy (v2, exploits dist/P symmetry + fp8 DoubleRow):
  - x/h loaded via SWDGE casting DMAs (f32 -> bf16).  x transposed on
    TensorE (bf16, PSUM) then cast to fp8e4 xT [128,2,S]; h transposed
    by the DMA xbar (dma_start_transpose) straight into SBUF bf16.
  - gram computed ONLY for the upper block-triangle (j >= 128*i) as
    fp8e4 DoubleRow matmuls (K=256 in one instruction); the
    "-0.5*|x_j|^2" row is accumulated via a bf16 K=8 block-diag aug
    matmul; |x_t|^2+MARGIN is the sqrt activation's per-partition bias
    (scale=-2), replacing max(d2,0)+eps (shift cancels in softmax).
  - sqrt/exp run only on the upper triangle (36/64 blocks); the lower
    P blocks are exp'd upper blocks transposed by the DMA xbar (P is
    symmetric), costing no PE/ACT/DVE time.
  - softmax skips max-subtraction and b_sim (cancel in the normalize);
    denominators come from a ones-column in the PV rhs.
  - gate folded into PV: Z = tanh((P@hW1)/den + h@W2' + bg) with
    hW = h @ [W1|W2]^T computed per batch, bg added via a K=1 aug.
  - ScalarE table-set discipline: all Sqrt precede all Exp/Tanh.
  - a ~4us dummy-matmul burst at kernel start trips the PE HAM clock
    gate to 8/8 under the initial DMAs.
"""

import sys

import numpy as np

if "/opt/trn_rl_repo" not in sys.path:
    sys.path.append("/opt/trn_rl_repo")

import concourse.bacc as bacc
import concourse.bass as bass
import concourse.mybir as mybir
import concourse.tile as tile
from concourse.bass import ts
from concourse.bass_utils import run_bass_kernel_spmd
from concourse.masks import make_identity

F32 = mybir.dt.float32
BF16 = mybir.dt.bfloat16
FP8 = mybir.dt.float8e4
AF = mybir.ActivationFunctionType
OP = mybir.AluOpType
PM = mybir.MatmulPerfMode

S = 1024
B = 32
NCORES = 8
BS = B // NCORES  # batches per core
E = 256
H = 256
A = 256
NT = S // 128  # 8 t-tiles
MARGIN = 12.0  # replaces max(d2,0)+eps; absorbs fp8 gram rounding (shift ~cancels in softmax)


def build_graph():
    nc = bacc.Bacc("TRN2", target_bir_lowering=False, debug=False)

    x_ext = nc.declare_dram_parameter("x", [S, BS, E], F32, isOutput=False)
    h_ext = nc.declare_dram_parameter("h", [S, BS, H], F32, isOutput=False)
    w_ext = nc.declare_dram_parameter("w_sim", [1, 1], F32, isOutput=False)
    wg_ext = nc.declare_dram_parameter("W_g", [A, 2 * H], F32, isOutput=False)
    bg_ext = nc.declare_dram_parameter("b_g", [1, A], F32, isOutput=False)
    out_ext = nc.declare_dram_parameter("out", [S, BS, A], F32, isOutput=True)

    with tile.TileContext(nc) as tc:
        with (
            tc.tile_pool(name="consts", bufs=1) as consts,
            tc.tile_pool(name="dist", bufs=BS) as distp,
            tc.tile_pool(name="work", bufs=2) as work,
            tc.tile_pool(name="nat", bufs=4) as natp,
            tc.tile_pool(name="small", bufs=2) as smallp,
            tc.tile_pool(name="zt", bufs=3) as ztp,
            tc.tile_pool(name="ps_bigb", bufs=2, space="PSUM") as psbb,
            tc.tile_pool(name="ps_d2", bufs=2, space="PSUM") as psd,
            tc.tile_pool(name="ps_f32", bufs=2, space="PSUM") as psf,
        ):
            # PE HAM warm-up: depends only on one fast DVE memset, so the
            # dense matmul burst starts ~immediately and trips the clock
            # gate to 8/8 while the input DMAs are still in flight.
            warm_in = consts.tile([128, 128], BF16)
            nc.vector.memset(warm_in, 1.0)
            warm_ps = psf.tile([128, 512], F32, tag="big")
            for _ in range(44):
                nc.tensor.matmul(
                    warm_ps[:, 0:128], warm_in[:], warm_in[:], start=True, stop=True
                )

            # prefetch x then h (SWDGE casting DMAs); x first so phase 1
            # of batch 0 can start as early as possible.
            xnat_list = []
            for b in range(BS):
                xnat = natp.tile([128, NT, E], BF16, tag="xnat")
                xnat_list.append(xnat)
                nc.gpsimd.dma_start(
                    out=xnat,
                    in_=x_ext[:, b, :].rearrange("(i p) e -> p i e", p=128),
                )
            hnat_list = []
            for b in range(BS):
                hnat = natp.tile([128, NT, H], BF16, tag="hnat")
                hnat_list.append(hnat)
                nc.gpsimd.dma_start(
                    out=hnat,
                    in_=h_ext[:, b, :].rearrange("(i p) e -> p i e", p=128),
                )

            # ---------------- constants ----------------
            ident = consts.tile([128, 128], F32)
            make_identity(nc, ident)
            identb = consts.tile([128, 128], BF16)
            nc.vector.tensor_copy(identb, ident)
            ones_stage = consts.tile([8, 128], F32)
            nc.vector.memset(ones_stage, 1.0)
            ones_row = consts.tile([1, 128], BF16)
            nc.vector.tensor_copy(ones_row, ones_stage[0:1, :])
            ones8 = consts.tile([8, 128], BF16)
            nc.vector.tensor_copy(ones8, ones_stage)
            zero_stage = consts.tile([8, S], F32)
            nc.vector.memset(zero_stage, 0.0)

            w_col = consts.tile([128, 1], F32)
            nc.sync.dma_start(out=w_col, in_=w_ext[:].partition_broadcast(128))
            bg_stage = consts.tile([1, A], F32)
            nc.sync.dma_start(out=bg_stage, in_=bg_ext[:])
            bg_row = consts.tile([1, A], BF16)
            nc.vector.tensor_copy(bg_row, bg_stage)

            # W_g (A, 2H) -> W12T: 2 k-tiles of (128hd, [A | A]) used as hW rhs
            wnat = consts.tile([128, 2, 2 * H], F32)
            nc.sync.dma_start(
                out=wnat, in_=wg_ext[:].rearrange("(m p) k -> p m k", m=2)
            )
            w12t = consts.tile([128, 2, 2 * H], BF16)
            for k2 in range(2):
                ps = psf.tile([128, 512], F32, tag="big")
                for w in range(2):
                    for m in range(2):
                        nc.tensor.transpose(
                            ps[:, w * 256 + m * 128 : w * 256 + (m + 1) * 128],
                            wnat[:, m, w * 256 + k2 * 128 : w * 256 + (k2 + 1) * 128],
                            ident[:],
                        )
                nc.vector.tensor_copy(w12t[:, k2, :], ps[:])

            # ---------------- phase 1: distances (upper triangle) ----------------
            d_tiles = []
            sqrt_instrs = []
            for b in range(BS):
                xT8 = work.tile([128, 2, S], FP8, tag="xT")
                sqmcol = smallp.tile([128, NT], F32, tag="sqm")
                biasp = smallp.tile([128, NT], F32, tag="bias")
                blockdiag = smallp.tile([8, S], BF16, tag="bd")
                nc.vector.tensor_copy(blockdiag, zero_stage)
                d_b = distp.tile([128, NT, S], BF16, tag="D")
                d_tiles.append(d_b)

                xnat = xnat_list[b]

                # transpose pairs of x tiles: psum [T0e0|T0e1|T1e0|T1e1],
                # cast to fp8 on the PSUM->SBUF copy
                for p2 in range(NT // 2):
                    i0, i1 = 2 * p2, 2 * p2 + 1
                    ps = psbb.tile([128, 512], BF16, tag="bigb")
                    for t2, i in enumerate((i0, i1)):
                        for k2 in range(2):
                            nc.tensor.transpose(
                                ps[:, t2 * 256 + k2 * 128 : t2 * 256 + (k2 + 1) * 128],
                                xnat[:, i, ts(k2, 128)],
                                identb[:],
                            )
                    # dst (k2, t2, f) ; src (t2, k2, f)
                    nc.vector.tensor_copy(
                        xT8[:, :, i0 * 128 : i0 * 128 + 256].rearrange(
                            "p k (t f) -> p t k f", t=2
                        ),
                        ps[:].rearrange("p (t k f) -> p t k f", t=2, k=2),
                    )

                # sqmcol[:, i] = |x_t|^2 per-partition, per t-tile (DVE)
                for i in range(NT):
                    scr = smallp.tile([128, E], F32, tag="scr")
                    nc.vector.scalar_tensor_tensor(
                        out=scr,
                        in0=xnat[:, i, :],
                        scalar=1.0,
                        in1=xnat[:, i, :],
                        op0=OP.mult,
                        op1=OP.mult,
                        accum_out=sqmcol[:, i : i + 1],
                    )
                # bias = |x_t|^2 + MARGIN
                nc.vector.tensor_scalar_add(out=biasp, in0=sqmcol, scalar1=MARGIN)
                # block-diagonal (8, S) holding -0.5*|x_j|^2
                sqmb = smallp.tile([128, NT], BF16, tag="sqmb")
                nc.vector.tensor_copy(sqmb[:], sqmcol[:])
                sq8 = psbb.tile([8, 128], BF16, tag="bigb")
                nc.tensor.transpose(sq8[:], sqmb[:], identb[:])
                sq8sb = smallp.tile([8, 128], BF16, tag="sq8sb")
                nc.vector.tensor_scalar_mul(sq8sb[:], sq8[:], -0.5)
                # scatter row k to blockdiag[k, 128k:128(k+1)] via a strided DMA
                bd = blockdiag[:]
                diag_view = bass.AP(
                    tensor=bd.tensor, offset=bd.offset, ap=[[S + 128, NT], [1, 128]]
                )
                nc.sync.dma_start(out=diag_view, in_=sq8sb[:])

                for i in range(NT):
                    lo = 128 * i
                    d2 = psd.tile([128, S], F32, tag="d2")
                    chunks = [(lo, 512), (512, S)] if lo < 512 else [(lo, S)]
                    # DoubleRow gram (K=256 in one mm), then the bf16 aug
                    for c0, c1 in chunks:
                        nc.tensor.matmul(
                            d2[:, c0:c1],
                            xT8[:, :, lo : lo + 128],
                            xT8[:, :, c0:c1],
                            start=True,
                            stop=False,
                            perf_mode=PM.DoubleRow,
                        )
                        nc.tensor.matmul(
                            d2[:, c0:c1],
                            ones8[:],
                            blockdiag[:, c0:c1],
                            start=False,
                            stop=True,
                        )
                    # dist = sqrt(-2*psum + |x_t|^2 + MARGIN), upper cols only
                    si = nc.scalar.activation(
                        out=d_b[:, i, lo:S],
                        in_=d2[:, lo:S],
                        func=AF.Sqrt,
                        bias=biasp[:, i : i + 1],
                        scale=-2.0,
                    )
                    sqrt_instrs.append(si)

            # ---------------- phase 2: softmax + PV + gate ----------------
            for b in range(BS):
                hT = work.tile([128, 2, S], BF16, tag="hT")
                hw = work.tile([128, NT, 520], BF16, tag="hw")
                p_b = work.tile([128, NT, S], BF16, tag="P")

                hnat = hnat_list[b]
                # h^T via the DMA xbar, straight into SBUF
                for k2 in range(2):
                    for i in range(NT):
                        nc.sync.dma_start_transpose(
                            hT[:, k2, ts(i, 128)], hnat[:, i, ts(k2, 128)]
                        )

                # hW = h @ [W1|W2]^T (+ bg on the W2 half)
                for m in range(NT):
                    ps = psf.tile([128, 512], F32, tag="big")
                    nc.tensor.matmul(
                        ps[:],
                        hT[:, 0, ts(m, 128)],
                        w12t[:, 0, :],
                        start=True,
                        stop=False,
                    )
                    nc.tensor.matmul(
                        ps[:],
                        hT[:, 1, ts(m, 128)],
                        w12t[:, 1, :],
                        start=False,
                        stop=False,
                    )
                    nc.tensor.matmul(
                        ps[:, 256:512],
                        ones_row[:],
                        bg_row[:],
                        start=False,
                        stop=True,
                    )
                    hwm = hw[:, m, :]
                    dst = bass.AP(
                        tensor=hwm.tensor,
                        offset=hwm.offset,
                        ap=[hwm.ap[0], [257, 2], [1, 256]],
                    )
                    nc.vector.tensor_copy(
                        dst, ps[:].rearrange("p (u f) -> p u f", u=2)
                    )

                nc.vector.memset(hw[:, :, 256:257], 1.0)
                # P = exp(w * dist) on the upper triangle; lower blocks are
                # DMA-xbar transposes of the upper ones (P is symmetric).
                for i in range(NT):
                    lo = 128 * i
                    ei = nc.scalar.activation(
                        out=p_b[:, i, lo:S],
                        in_=d_tiles[b][:, i, lo:S],
                        func=AF.Exp,
                        scale=w_col[:, 0:1],
                    )
                    for si in sqrt_instrs:
                        tile.add_dep_helper(
                            ei.ins, si.ins, sync=False, reason="act-table-order"
                        )
                    for k in range(i + 1, NT):
                        nc.sync.dma_start_transpose(
                            p_b[:, k, ts(i, 128)], p_b[:, i, ts(k, 128)]
                        )

                for i2 in range(0, NT, 2):
                    zs = ztp.tile([128, 2, A], F32, tag="zs")
                    for u in range(2):
                        i = i2 + u
                        pv = psf.tile([128, 512], F32, tag="big")
                        for k in range(NT):
                            nc.tensor.matmul(
                                pv[:, 0 : A + 1],
                                p_b[:, k, ts(i, 128)],
                                hw[:, k, 0 : A + 1],
                                start=(k == 0),
                                stop=(k == NT - 1),
                            )
                        rp_i = smallp.tile([128, 1], F32, tag="rp_i")
                        nc.vector.reciprocal(rp_i[:], pv[:, A : A + 1])
                        nc.vector.scalar_tensor_tensor(
                            out=zs[:, u, :],
                            in0=pv[:, 0:A],
                            scalar=rp_i[:, 0:1],
                            in1=hw[:, i, 257 : 257 + A],
                            op0=OP.mult,
                            op1=OP.add,
                        )
                    zo = ztp.tile([128, 2, A], F32, tag="zo")
                    nc.scalar.activation(
                        out=zo[:].rearrange("p a b -> p (a b)"),
                        in_=zs[:].rearrange("p a b -> p (a b)"),
                        func=AF.Tanh,
                    )
                    nc.sync.dma_start(
                        out=out_ext[i2 * 128 : i2 * 128 + 256, b, :].rearrange(
                            "(u p) a -> p u a", p=128
                        ),
                        in_=zo,
                    )

    nc.compile()
    return nc


_CACHED = {}


def _get_graph():
    if "nc" not in _CACHED:
        _CACHED["nc"] = build_graph()
    return _CACHED["nc"]


def _run(inputs, trace=False, **kw):
    nc = _get_graph()
    x = np.asarray(inputs["x"], dtype=np.float32)
    h = np.asarray(inputs["h"], dtype=np.float32)
    w_sim = np.asarray(inputs["w_sim"], dtype=np.float32).reshape(1, 1)
    W_g = np.ascontiguousarray(np.asarray(inputs["W_g"], dtype=np.float32))
    b_g = np.asarray(inputs["b_g"], dtype=np.float32).reshape(1, A)
    in_maps = []
    for c in range(NCORES):
        in_maps.append(
            {
                "x": np.ascontiguousarray(x[:, c * BS : (c + 1) * BS, :]),
                "h": np.ascontiguousarray(h[:, c * BS : (c + 1) * BS, :]),
                "w_sim": w_sim,
                "W_g": W_g,
                "b_g": b_g,
            }
        )
    res = run_bass_kernel_spmd(nc, in_maps, list(range(NCORES)), trace=trace, **kw)
    out = np.concatenate([res.results[c]["out"] for c in range(NCORES)], axis=1)
    return out, res


def kernel(**inputs):
    out, _ = _run(inputs, trace=False)
    return out


if __name__ == "__main__":
    rng = np.random.default_rng(0)
    ins = {
        "x": rng.standard_normal((S, B, E), dtype=np.float32),
        "h": rng.standard_normal((S, B, H), dtype=np.float32),
        "w_sim": np.array([0.03], dtype=np.float32),
        "b_sim": np.array([0.01], dtype=np.float32),
        "W_g": (rng.standard_normal((A, 2 * H)) * 0.05).astype(np.float32),
        "b_g": np.zeros(A, dtype=np.float32),
    }
    out = kernel(**ins)
    print("out", out.shape, out.dtype, np.abs(out).mean())


# revision 6
# speedup vs baseline: 1.3509x; 1.3509x over previous
"""Distance-based attention (nn_Attention_67989332296336) on 8 TRN2 NeuronCores.

Math per batch element b (S=1024, E=H=A=256):
    d2[t,j]  = |x_t|^2 + |x_j|^2 - 2 x_t.x_j
    dist     = sqrt(max(d2,0)+eps)
    scores   = w_sim*dist + b_sim
    A        = softmax_j(scores)
    G        = A @ h
    Z        = tanh([G, h] @ W_g^T + b_g)

Sharding: batch dim B=32 split over 8 cores (4 per core), weights replicated.

Per-core strategy (v3: dist symmetry + fp8 DoubleRow gram/PV):
  - x/h loaded via SWDGE casting DMAs (f32 -> bf16).  x^T built on
    TensorE (bf16 PSUM) and cast to fp8e4 on the DVE evac; h^T for two
    batches via the sync-ring DMA-xbar, the other two on TensorE.
  - gram only for the upper block-triangle as fp8e4 DoubleRow matmuls
    (K=256/instr); "-0.5|x_j|^2" via a bf16 K=8 block-diag aug matmul;
    |x_t|^2+MARGIN is the sqrt bias (scale=-2).  sqrt runs per t-tile
    on the upper cols only; lower dist blocks are TensorE transposes of
    upper ones (dist is symmetric) evacuated by the DVE.
  - exp runs full-width in 4-row chunks, writing P straight to fp8e4;
    PV is DoubleRow fp8 (P pairs x hw1 pairs), with a ones column in
    hw1 giving softmax denominators.  Z = tanh(PV/den + hW2 + bg).
  - hW = h @ [W1|W2]^T per batch; the W1 half is cast to fp8 (DVE and
    ScalarE alternating), the W2 half to bf16 on the DVE; bg added via
    a K=1 aug matmul.
  - ScalarE table discipline: all Sqrt precede all Exp/Tanh.
  - a ~4us dummy-matmul burst trips the PE HAM clock gate under the
    initial DMAs.
"""

import sys

import numpy as np

if "/opt/trn_rl_repo" not in sys.path:
    sys.path.append("/opt/trn_rl_repo")

import concourse.bacc as bacc
import concourse.bass as bass
import concourse.mybir as mybir
import concourse.tile as tile
from concourse.bass import ts
from concourse.bass_utils import run_bass_kernel_spmd
from concourse.masks import make_identity

F32 = mybir.dt.float32
BF16 = mybir.dt.bfloat16
FP8 = mybir.dt.float8e4
AF = mybir.ActivationFunctionType
OP = mybir.AluOpType
PM = mybir.MatmulPerfMode

S = 1024
B = 32
NCORES = 8
BS = B // NCORES  # batches per core
E = 256
H = 256
A = 256
NT = S // 128  # 8 t-tiles
MARGIN = 12.0  # replaces max(d2,0)+eps; absorbs fp8 gram rounding (~cancels in softmax)
HW1P = 272  # hw1 pitch (257 used, padded to %16 for DoubleRow pair strides


def build_graph():
    nc = bacc.Bacc("TRN2", target_bir_lowering=False, debug=False)

    x_ext = nc.declare_dram_parameter("x", [S, BS, E], F32, isOutput=False)
    h_ext = nc.declare_dram_parameter("h", [S, BS, H], F32, isOutput=False)
    w_ext = nc.declare_dram_parameter("w_sim", [1, 1], F32, isOutput=False)
    wg_ext = nc.declare_dram_parameter("W_g", [A, 2 * H], F32, isOutput=False)
    bg_ext = nc.declare_dram_parameter("b_g", [1, A], F32, isOutput=False)
    out_ext = nc.declare_dram_parameter("out", [S, BS, A], F32, isOutput=True)

    with tile.TileContext(nc) as tc:
        with (
            tc.tile_pool(name="consts", bufs=1) as consts,
            tc.tile_pool(name="dist", bufs=BS) as distp,
            tc.tile_pool(name="work", bufs=2) as work,
            tc.tile_pool(name="nat", bufs=4) as natp,
            tc.tile_pool(name="small", bufs=2) as smallp,
            tc.tile_pool(name="zt", bufs=3) as ztp,
            tc.tile_pool(name="ps_bigb", bufs=2, space="PSUM") as psbb,
            tc.tile_pool(name="ps_d2", bufs=2, space="PSUM") as psd,
            tc.tile_pool(name="ps_f32", bufs=2, space="PSUM") as psf,
        ):
            # PE HAM warm-up burst under the initial DMAs
            warm_in = consts.tile([128, 128], BF16)
            nc.vector.memset(warm_in, 1.0)
            warm_ps = psf.tile([128, 512], F32, tag="big")
            for _ in range(44):
                nc.tensor.matmul(
                    warm_ps[:, 0:128], warm_in[:], warm_in[:], start=True, stop=True
                )

            # prefetch x (phase 1) then h (phase 2), batch-major
            xnat_list = []
            for b in range(BS):
                xnat = natp.tile([128, NT, E], BF16, tag="xnat")
                xnat_list.append(xnat)
                nc.gpsimd.dma_start(
                    out=xnat,
                    in_=x_ext[:, b, :].rearrange("(i p) e -> p i e", p=128),
                )
            hnat_list = []
            for b in range(BS):
                hnat = natp.tile([128, NT, H], BF16, tag="hnat")
                hnat_list.append(hnat)
                nc.gpsimd.dma_start(
                    out=hnat,
                    in_=h_ext[:, b, :].rearrange("(i p) e -> p i e", p=128),
                )

            # ---------------- constants ----------------
            ident = consts.tile([128, 128], F32)
            make_identity(nc, ident)
            identb = consts.tile([128, 128], BF16)
            nc.vector.tensor_copy(identb, ident)
            ones_stage = consts.tile([8, 128], F32)
            nc.vector.memset(ones_stage, 1.0)
            ones_row = consts.tile([1, 128], BF16)
            nc.vector.tensor_copy(ones_row, ones_stage[0:1, :])
            ones8 = consts.tile([8, 128], BF16)
            nc.vector.tensor_copy(ones8, ones_stage)
            zero_stage = consts.tile([8, S], F32)
            nc.vector.memset(zero_stage, 0.0)

            w_col = consts.tile([128, 1], F32)
            nc.sync.dma_start(out=w_col, in_=w_ext[:].partition_broadcast(128))
            bg_stage = consts.tile([1, A], F32)
            nc.sync.dma_start(out=bg_stage, in_=bg_ext[:])
            bg_row = consts.tile([1, A], BF16)
            nc.vector.tensor_copy(bg_row, bg_stage)

            # W_g (A, 2H) -> W12T: 2 k-tiles of (128hd, [A | A]) used as hW rhs
            wnat = consts.tile([128, 2, 2 * H], F32)
            nc.sync.dma_start(
                out=wnat, in_=wg_ext[:].rearrange("(m p) k -> p m k", m=2)
            )
            w12t = consts.tile([128, 2, 2 * H], BF16)
            for k2 in range(2):
                ps = psf.tile([128, 512], F32, tag="big")
                for w in range(2):
                    for m in range(2):
                        nc.tensor.transpose(
                            ps[:, w * 256 + m * 128 : w * 256 + (m + 1) * 128],
                            wnat[:, m, w * 256 + k2 * 128 : w * 256 + (k2 + 1) * 128],
                            ident[:],
                        )
                nc.vector.tensor_copy(w12t[:, k2, :], ps[:])

            # ---------------- phase 1: upper-triangle distances ----------------
            d_tiles = []
            sqrt_instrs = []
            for b in range(BS):
                xT8 = work.tile([128, 2, S], FP8, tag="xT")
                sqmcol = smallp.tile([128, NT], F32, tag="sqm")
                biasp = smallp.tile([128, NT], F32, tag="bias")
                blockdiag = smallp.tile([8, S], BF16, tag="bd")
                nc.vector.tensor_copy(blockdiag, zero_stage)
                d_b = distp.tile([128, NT, S], BF16, tag="D")
                d_tiles.append(d_b)

                xnat = xnat_list[b]

                # x^T: TensorE transpose pairs into bf16 PSUM, DVE evac casts
                # to fp8: psum holds [T0e0|T0e1|T1e0|T1e1]
                for p2 in range(NT // 2):
                    i0, i1 = 2 * p2, 2 * p2 + 1
                    ps = psbb.tile([128, 512], BF16, tag="bigb")
                    for t2, i in enumerate((i0, i1)):
                        for k2 in range(2):
                            nc.tensor.transpose(
                                ps[:, t2 * 256 + k2 * 128 : t2 * 256 + (k2 + 1) * 128],
                                xnat[:, i, ts(k2, 128)],
                                identb[:],
                            )
                    nc.vector.tensor_copy(
                        xT8[:, :, i0 * 128 : i0 * 128 + 256].rearrange(
                            "p k (t f) -> p t k f", t=2
                        ),
                        ps[:].rearrange("p (t k f) -> p t k f", t=2, k=2),
                    )

                # |x_t|^2 per t-tile (DVE); bias = |x_t|^2 + MARGIN
                for i in range(NT):
                    scr = smallp.tile([128, E], F32, tag="scr")
                    nc.vector.scalar_tensor_tensor(
                        out=scr,
                        in0=xnat[:, i, :],
                        scalar=1.0,
                        in1=xnat[:, i, :],
                        op0=OP.mult,
                        op1=OP.mult,
                        accum_out=sqmcol[:, i : i + 1],
                    )
                nc.vector.tensor_scalar_add(out=biasp, in0=sqmcol, scalar1=MARGIN)
                # block-diagonal (8, S) holding -0.5*|x_j|^2
                sqmb = smallp.tile([128, NT], BF16, tag="sqmb")
                nc.vector.tensor_copy(sqmb[:], sqmcol[:])
                sq8 = psbb.tile([8, 128], BF16, tag="bigb")
                nc.tensor.transpose(sq8[:], sqmb[:], identb[:])
                sq8sb = smallp.tile([8, 128], BF16, tag="sq8sb")
                nc.vector.tensor_scalar_mul(sq8sb[:], sq8[:], -0.5)
                bd = blockdiag[:]
                diag_view = bass.AP(
                    tensor=bd.tensor, offset=bd.offset, ap=[[S + 128, NT], [1, 128]]
                )
                nc.sync.dma_start(out=diag_view, in_=sq8sb[:])

                for i in range(NT):
                    lo = 128 * i
                    d2 = psd.tile([128, S], F32, tag="d2")
                    chunks = [(lo, 512), (512, S)] if lo < 512 else [(lo, S)]
                    for c0, c1 in chunks:
                        nc.tensor.matmul(
                            d2[:, c0:c1],
                            xT8[:, :, lo : lo + 128],
                            xT8[:, :, c0:c1],
                            start=True,
                            stop=False,
                            perf_mode=PM.DoubleRow,
                        )
                        nc.tensor.matmul(
                            d2[:, c0:c1],
                            ones8[:],
                            blockdiag[:, c0:c1],
                            start=False,
                            stop=True,
                        )
                    # dist = sqrt(-2*psum + |x_t|^2 + MARGIN) on upper cols
                    si = nc.scalar.activation(
                        out=d_b[:, i, lo:S],
                        in_=d2[:, lo:S],
                        func=AF.Sqrt,
                        bias=biasp[:, i : i + 1],
                        scale=-2.0,
                    )
                    sqrt_instrs.append(si)
                    # lower blocks (k, i), k>i = transposes of row i's upper
                    # blocks; TensorE -> bf16 PSUM -> one DVE evac copy
                    if i < NT - 1:
                        nb = NT - 1 - i
                        tp = psbb.tile([128, 1024], BF16, tag="bigb")
                        for u in range(nb):
                            nc.tensor.transpose(
                                tp[:, u * 128 : (u + 1) * 128],
                                d_b[:, i, (i + 1 + u) * 128 : (i + 2 + u) * 128],
                                identb[:],
                            )
                        nc.vector.tensor_copy(
                            d_b[:, i + 1 :, ts(i, 128)],
                            tp[:, 0 : nb * 128].rearrange(
                                "p (u f) -> p u f", u=nb
                            ),
                        )

            # ---------------- phase 2: softmax + PV + gate ----------------
            for b in range(BS):
                hT = work.tile([128, 2, S], BF16, tag="hT")
                hw1 = work.tile([128, NT, HW1P], FP8, tag="hw1")
                hw2 = work.tile([128, NT, A], BF16, tag="hw2")
                p8 = work.tile([128, NT, S], FP8, tag="P")

                hnat = hnat_list[b]
                if b < 2:
                    # h^T via the sync-ring DMA xbar (ring has slack for 2 batches)
                    for k2 in range(2):
                        for i in range(NT):
                            nc.sync.dma_start_transpose(
                                hT[:, k2, ts(i, 128)], hnat[:, i, ts(k2, 128)]
                            )
                else:
                    # h^T on TensorE like x^T, bf16 evac
                    for p2 in range(NT // 2):
                        i0, i1 = 2 * p2, 2 * p2 + 1
                        ps = psbb.tile([128, 512], BF16, tag="bigb")
                        for t2, i in enumerate((i0, i1)):
                            for k2 in range(2):
                                nc.tensor.transpose(
                                    ps[
                                        :,
                                        t2 * 256
                                        + k2 * 128 : t2 * 256
                                        + (k2 + 1) * 128,
                                    ],
                                    hnat[:, i, ts(k2, 128)],
                                    identb[:],
                                )
                        nc.vector.tensor_copy(
                            hT[:, :, i0 * 128 : i0 * 128 + 256].rearrange(
                                "p k (t f) -> p t k f", t=2
                            ),
                            ps[:].rearrange("p (t k f) -> p t k f", t=2, k=2),
                        )

                # hW = h @ [W1|W2]^T (+ bg on the W2 half)
                for m in range(NT):
                    ps = psf.tile([128, 512], F32, tag="big")
                    nc.tensor.matmul(
                        ps[:],
                        hT[:, 0, ts(m, 128)],
                        w12t[:, 0, :],
                        start=True,
                        stop=False,
                    )
                    nc.tensor.matmul(
                        ps[:],
                        hT[:, 1, ts(m, 128)],
                        w12t[:, 1, :],
                        start=False,
                        stop=False,
                    )
                    nc.tensor.matmul(
                        ps[:, 256:512],
                        ones_row[:],
                        bg_row[:],
                        start=False,
                        stop=True,
                    )
                    # W1 half -> fp8 (alternate DVE/ScalarE), W2 half -> bf16
                    if m % 2 == 0:
                        nc.vector.tensor_copy(hw1[:, m, 0:256], ps[:, 0:256])
                    else:
                        nc.scalar.copy(hw1[:, m, 0:256], ps[:, 0:256])
                    nc.vector.tensor_copy(hw2[:, m, :], ps[:, 256:512])

                nc.vector.memset(hw1[:, :, 256:257], 1.0)
                # P = exp(w * dist), full rows, straight to fp8
                for i4 in range(0, NT, 4):
                    ei = nc.scalar.activation(
                        out=p8[:, i4 : i4 + 4, :],
                        in_=d_tiles[b][:, i4 : i4 + 4, :],
                        func=AF.Exp,
                        scale=w_col[:, 0:1],
                    )
                    for si in sqrt_instrs:
                        tile.add_dep_helper(
                            ei.ins, si.ins, sync=False, reason="act-table-order"
                        )

                for i4 in range(0, NT, 4):
                    zs = ztp.tile([128, 4, A], F32, tag="zs")
                    for u in range(4):
                        i = i4 + u
                        pv = psf.tile([128, 512], F32, tag="big")
                        for k2 in range(0, NT, 2):
                            nc.tensor.matmul(
                                pv[:, 0 : A + 1],
                                p8[:, k2 : k2 + 2, ts(i, 128)],
                                hw1[:, k2 : k2 + 2, 0 : A + 1],
                                start=(k2 == 0),
                                stop=(k2 == NT - 2),
                                perf_mode=PM.DoubleRow,
                            )
                        rp_i = smallp.tile([128, 1], F32, tag="rp_i")
                        nc.vector.reciprocal(rp_i[:], pv[:, A : A + 1])
                        nc.vector.scalar_tensor_tensor(
                            out=zs[:, u, :],
                            in0=pv[:, 0:A],
                            scalar=rp_i[:, 0:1],
                            in1=hw2[:, i, :],
                            op0=OP.mult,
                            op1=OP.add,
                        )
                    zo = ztp.tile([128, 4, A], F32, tag="zo")
                    nc.scalar.activation(
                        out=zo[:].rearrange("p a b -> p (a b)"),
                        in_=zs[:].rearrange("p a b -> p (a b)"),
                        func=AF.Tanh,
                    )
                    nc.sync.dma_start(
                        out=out_ext[i4 * 128 : i4 * 128 + 512, b, :].rearrange(
                            "(u p) a -> p u a", p=128
                        ),
                        in_=zo,
                    )

    nc.compile()
    return nc


_CACHED = {}


def _get_graph():
    if "nc" not in _CACHED:
        _CACHED["nc"] = build_graph()
    return _CACHED["nc"]


def _run(inputs, trace=False, **kw):
    nc = _get_graph()
    x = np.asarray(inputs["x"], dtype=np.float32)
    h = np.asarray(inputs["h"], dtype=np.float32)
    w_sim = np.asarray(inputs["w_sim"], dtype=np.float32).reshape(1, 1)
    W_g = np.ascontiguousarray(np.asarray(inputs["W_g"], dtype=np.float32))
    b_g = np.asarray(inputs["b_g"], dtype=np.float32).reshape(1, A)
    in_maps = []
    for c in range(NCORES):
        in_maps.append(
            {
                "x": np.ascontiguousarray(x[:, c * BS : (c + 1) * BS, :]),
                "h": np.ascontiguousarray(h[:, c * BS : (c + 1) * BS, :]),
                "w_sim": w_sim,
                "W_g": W_g,
                "b_g": b_g,
            }
        )
    res = run_bass_kernel_spmd(nc, in_maps, list(range(NCORES)), trace=trace, **kw)
    out = np.concatenate([res.results[c]["out"] for c in range(NCORES)], axis=1)
    return out, res


def kernel(**inputs):
    out, _ = _run(inputs, trace=False)
    return out


if __name__ == "__main__":
    rng = np.random.default_rng(0)
    ins = {
        "x": rng.standard_normal((S, B, E), dtype=np.float32),
        "h": rng.standard_normal((S, B, E), dtype=np.float32),
        "w_sim": np.array([0.03], dtype=np.float32),
        "b_sim": np.array([0.01], dtype=np.float32),
        "W_g": (rng.standard_normal((A, 2 * H)) * 0.05).astype(np.float32),
        "b_g": np.zeros(A, dtype=np.float32),
    }
    out = kernel(**ins)
    print("out", out.shape, out.dtype, np.abs(out).mean())


# revision 13
# speedup vs baseline: 1.4347x; 1.0620x over previous
"""Distance-based attention (nn_Attention_67989332296336) on 8 TRN2 NeuronCores.

Math per batch element b (S=1024, E=H=A=256):
    d2[t,j]  = |x_t|^2 + |x_j|^2 - 2 x_t.x_j
    dist     = sqrt(max(d2,0)+eps)
    scores   = w_sim*dist + b_sim
    A        = softmax_j(scores)
    G        = A @ h
    Z        = tanh([G, h] @ W_g^T + b_g)

Sharding: batch dim B=32 split over 8 cores (4 per core), weights replicated.

Per-core strategy (v3: dist symmetry + fp8 DoubleRow gram/PV):
  - x/h loaded via SWDGE casting DMAs (f32 -> bf16).  x^T built on
    TensorE (bf16 PSUM) and cast to fp8e4 on the DVE evac; h^T for two
    batches via the sync-ring DMA-xbar, the other two on TensorE.
  - gram only for the upper block-triangle as fp8e4 DoubleRow matmuls
    (K=256/instr); "-0.5|x_j|^2" via a bf16 K=8 block-diag aug matmul;
    |x_t|^2+MARGIN is the sqrt bias (scale=-2).  sqrt runs per t-tile
    on the upper cols only; lower dist blocks are TensorE transposes of
    upper ones (dist is symmetric) evacuated by the DVE.
  - exp runs full-width in 4-row chunks, writing P straight to fp8e4;
    PV is DoubleRow fp8 (P pairs x hw1 pairs), with a ones column in
    hw1 giving softmax denominators.  Z = tanh(PV/den + hW2 + bg).
  - hW = h @ [W1|W2]^T per batch; the W1 half is cast to fp8 (DVE and
    ScalarE alternating), the W2 half to bf16 on the DVE; bg added via
    a K=1 aug matmul.
  - ScalarE table discipline: all Sqrt precede all Exp/Tanh.
  - a ~4us dummy-matmul burst trips the PE HAM clock gate under the
    initial DMAs.
"""

import sys

import numpy as np

if "/opt/trn_rl_repo" not in sys.path:
    sys.path.append("/opt/trn_rl_repo")

import concourse.bacc as bacc
import concourse.bass as bass
import concourse.mybir as mybir
import concourse.tile as tile
from concourse.bass import ts
from concourse.bass_utils import run_bass_kernel_spmd
from concourse.masks import make_identity

F32 = mybir.dt.float32
BF16 = mybir.dt.bfloat16
FP8 = mybir.dt.float8e4
AF = mybir.ActivationFunctionType
OP = mybir.AluOpType
PM = mybir.MatmulPerfMode

S = 1024
B = 32
NCORES = 8
BS = B // NCORES  # batches per core
E = 256
H = 256
A = 256
NT = S // 128  # 8 t-tiles
MARGIN = 12.0  # replaces max(d2,0)+eps; absorbs fp8 gram rounding (~cancels in softmax)
HW1P = 272  # hw1 pitch (257 used, padded to %16 for DoubleRow pair strides


def build_graph():
    nc = bacc.Bacc("TRN2", target_bir_lowering=False, debug=False)

    x_ext = nc.declare_dram_parameter("x", [S, BS, E], F32, isOutput=False)
    h_ext = nc.declare_dram_parameter("h", [S, BS, H], F32, isOutput=False)
    w_ext = nc.declare_dram_parameter("w_sim", [1, 1], F32, isOutput=False)
    wg_ext = nc.declare_dram_parameter("W_g", [A, 2 * H], F32, isOutput=False)
    bg_ext = nc.declare_dram_parameter("b_g", [1, A], F32, isOutput=False)
    out_ext = nc.declare_dram_parameter("out", [S, BS, A], F32, isOutput=True)

    with tile.TileContext(nc) as tc:
        with (
            tc.tile_pool(name="consts", bufs=1) as consts,
            tc.tile_pool(name="dist", bufs=BS) as distp,
            tc.tile_pool(name="work", bufs=2) as work,
            tc.tile_pool(name="nat", bufs=4) as natp,
            tc.tile_pool(name="small", bufs=2) as smallp,
            tc.tile_pool(name="zt", bufs=3) as ztp,
            tc.tile_pool(name="ps_bigb", bufs=2, space="PSUM") as psbb,
            tc.tile_pool(name="ps_d2", bufs=2, space="PSUM") as psd,
            tc.tile_pool(name="ps_f32", bufs=2, space="PSUM") as psf,
        ):
            # PE HAM warm-up burst under the initial DMAs
            warm_in = consts.tile([128, 128], BF16)
            nc.vector.memset(warm_in, 1.0)
            warm_ps = psf.tile([128, 512], F32, tag="big")
            for _ in range(44):
                nc.tensor.matmul(
                    warm_ps[:, 0:128], warm_in[:], warm_in[:], start=True, stop=True
                )

            # prefetch inputs; order chosen so phase-1 (x) and the sync-ring
            # h-transposes (h0/h1) are fed as early as possible:
            # x0, h0, x1, h1, x2, x3, h2, h3
            xnat_list = [None] * BS
            hnat_list = [None] * BS
            def load_x(b):
                xnat = natp.tile([128, NT, E], BF16, tag="xnat")
                xnat_list[b] = xnat
                nc.gpsimd.dma_start(
                    out=xnat,
                    in_=x_ext[:, b, :].rearrange("(i p) e -> p i e", p=128),
                )
            def load_h(b):
                hnat = natp.tile([128, NT, H], BF16, tag="hnat")
                hnat_list[b] = hnat
                nc.gpsimd.dma_start(
                    out=hnat,
                    in_=h_ext[:, b, :].rearrange("(i p) e -> p i e", p=128),
                )
            load_x(0); load_h(0); load_x(1); load_h(1)
            load_x(2); load_x(3); load_h(2); load_h(3)

            # ---------------- constants ----------------
            ident = consts.tile([128, 128], F32)
            make_identity(nc, ident)
            identb = consts.tile([128, 128], BF16)
            nc.vector.tensor_copy(identb, ident)
            ones_stage = consts.tile([8, 128], F32)
            nc.vector.memset(ones_stage, 1.0)
            ones_row = consts.tile([1, 128], BF16)
            nc.vector.tensor_copy(ones_row, ones_stage[0:1, :])
            ones8 = consts.tile([8, 128], BF16)
            nc.vector.tensor_copy(ones8, ones_stage)
            zero_stage = consts.tile([8, S], F32)
            nc.vector.memset(zero_stage, 0.0)

            w_col = consts.tile([128, 1], F32)
            nc.sync.dma_start(out=w_col, in_=w_ext[:].partition_broadcast(128))
            bg_stage = consts.tile([1, A], F32)
            nc.sync.dma_start(out=bg_stage, in_=bg_ext[:])
            bg_row = consts.tile([1, A], BF16)
            nc.vector.tensor_copy(bg_row, bg_stage)

            # W_g (A, 2H) -> W12T: 2 k-tiles of (128hd, [A | A]) used as hW rhs
            wnat = consts.tile([128, 2, 2 * H], F32)
            nc.sync.dma_start(
                out=wnat, in_=wg_ext[:].rearrange("(m p) k -> p m k", m=2)
            )
            w12t = consts.tile([128, 2, 2 * H], BF16)
            for k2 in range(2):
                ps = psf.tile([128, 512], F32, tag="big")
                for w in range(2):
                    for m in range(2):
                        nc.tensor.transpose(
                            ps[:, w * 256 + m * 128 : w * 256 + (m + 1) * 128],
                            wnat[:, m, w * 256 + k2 * 128 : w * 256 + (k2 + 1) * 128],
                            ident[:],
                        )
                nc.vector.tensor_copy(w12t[:, k2, :], ps[:])

            # ---------------- phase 1: upper-triangle distances ----------------
            d_tiles = []
            sqrt_instrs = []
            xt_transp_instrs = []  # PE x-transposes (order anchors)
            xt_evac_instrs = []  # DVE x-evacs (order anchors)
            for b in range(BS):
                xT8 = work.tile([128, 2, S], FP8, tag="xT")
                sqmcol = smallp.tile([128, NT], F32, tag="sqm")
                biasp = smallp.tile([128, NT], F32, tag="bias")
                blockdiag = smallp.tile([8, S], BF16, tag="bd")
                nc.vector.tensor_copy(blockdiag, zero_stage)
                d_b = distp.tile([128, NT, S], BF16, tag="D")
                d_tiles.append(d_b)

                xnat = xnat_list[b]

                # x^T: TensorE transpose pairs into bf16 PSUM, DVE evac casts
                # to fp8: psum holds [T0e0|T0e1|T1e0|T1e1]
                for p2 in range(NT // 2):
                    i0, i1 = 2 * p2, 2 * p2 + 1
                    ps = psbb.tile([128, 512], BF16, tag="bigb")
                    for t2, i in enumerate((i0, i1)):
                        for k2 in range(2):
                            ti = nc.tensor.transpose(
                                ps[:, t2 * 256 + k2 * 128 : t2 * 256 + (k2 + 1) * 128],
                                xnat[:, i, ts(k2, 128)],
                                identb[:],
                            )
                            xt_transp_instrs.append(ti)
                    ci = nc.vector.tensor_copy(
                        xT8[:, :, i0 * 128 : i0 * 128 + 256].rearrange(
                            "p k (t f) -> p t k f", t=2
                        ),
                        ps[:].rearrange("p (t k f) -> p t k f", t=2, k=2),
                    )
                    xt_evac_instrs.append(ci)

                # |x_t|^2 per t-tile (DVE); bias = |x_t|^2 + MARGIN
                for i in range(NT):
                    scr = smallp.tile([128, E], F32, tag="scr")
                    nc.vector.scalar_tensor_tensor(
                        out=scr,
                        in0=xnat[:, i, :],
                        scalar=1.0,
                        in1=xnat[:, i, :],
                        op0=OP.mult,
                        op1=OP.mult,
                        accum_out=sqmcol[:, i : i + 1],
                    )
                nc.vector.tensor_scalar_add(out=biasp, in0=sqmcol, scalar1=MARGIN)
                # block-diagonal (8, S) holding -0.5*|x_j|^2
                sqmb = smallp.tile([128, NT], BF16, tag="sqmb")
                nc.vector.tensor_copy(sqmb[:], sqmcol[:])
                sq8 = psbb.tile([8, 128], BF16, tag="bigb")
                nc.tensor.transpose(sq8[:], sqmb[:], identb[:])
                sq8sb = smallp.tile([8, 128], BF16, tag="sq8sb")
                nc.vector.tensor_scalar_mul(sq8sb[:], sq8[:], -0.5)
                bd = blockdiag[:]
                diag_view = bass.AP(
                    tensor=bd.tensor, offset=bd.offset, ap=[[S + 128, NT], [1, 128]]
                )
                nc.gpsimd.dma_start(out=diag_view, in_=sq8sb[:])

                for i in range(NT):
                    lo = 128 * i
                    d2 = psd.tile([128, S], F32, tag="d2")
                    chunks = [(lo, 512), (512, S)] if lo < 512 else [(lo, S)]
                    for c0, c1 in chunks:
                        nc.tensor.matmul(
                            d2[:, c0:c1],
                            xT8[:, :, lo : lo + 128],
                            xT8[:, :, c0:c1],
                            start=True,
                            stop=False,
                            perf_mode=PM.DoubleRow,
                        )
                        nc.tensor.matmul(
                            d2[:, c0:c1],
                            ones8[:],
                            blockdiag[:, c0:c1],
                            start=False,
                            stop=True,
                        )
                    # dist = sqrt(-2*psum + |x_t|^2 + MARGIN) on upper cols
                    si = nc.scalar.activation(
                        out=d_b[:, i, lo:S],
                        in_=d2[:, lo:S],
                        func=AF.Sqrt,
                        bias=biasp[:, i : i + 1],
                        scale=-2.0,
                    )
                    sqrt_instrs.append(si)
                    # lower blocks (k, i), k>i = transposes of row i's upper
                    # blocks; TensorE -> bf16 PSUM -> one DVE evac copy
                    if i < NT - 1:
                        nb = NT - 1 - i
                        tp = psbb.tile([128, 1024], BF16, tag="bigb")
                        for u in range(nb):
                            nc.tensor.transpose(
                                tp[:, u * 128 : (u + 1) * 128],
                                d_b[:, i, (i + 1 + u) * 128 : (i + 2 + u) * 128],
                                identb[:],
                            )
                        nc.vector.tensor_copy(
                            d_b[:, i + 1 :, ts(i, 128)],
                            tp[:, 0 : nb * 128].rearrange(
                                "p (u f) -> p u f", u=nb
                            ),
                        )

            # ---------------- phase 2: softmax + PV + gate ----------------
            for b in range(BS):
                hT = work.tile([128, 2, S], BF16, tag="hT")
                hw1 = work.tile([128, NT, HW1P], FP8, tag="hw1")
                hw2 = work.tile([128, NT, A], BF16, tag="hw2")
                p8 = work.tile([128, NT, S], FP8, tag="P")

                hnat = hnat_list[b]
                if b < 2:
                    # h^T via the sync-ring DMA xbar (ring has slack for 2 batches)
                    for k2 in range(2):
                        for i in range(NT):
                            nc.sync.dma_start_transpose(
                                hT[:, k2, ts(i, 128)], hnat[:, i, ts(k2, 128)]
                            )
                else:
                    # h^T on TensorE like x^T, bf16 evac.  Ordering deps keep
                    # these behind all phase-1 x work on the PE/DVE queues so
                    # a late h DMA cannot head-block either queue.
                    for p2 in range(NT // 2):
                        i0, i1 = 2 * p2, 2 * p2 + 1
                        ps = psbb.tile([128, 512], BF16, tag="bigb")
                        for t2, i in enumerate((i0, i1)):
                            for k2 in range(2):
                                ti = nc.tensor.transpose(
                                    ps[
                                        :,
                                        t2 * 256
                                        + k2 * 128 : t2 * 256
                                        + (k2 + 1) * 128,
                                    ],
                                    hnat[:, i, ts(k2, 128)],
                                    identb[:],
                                )
                                for xi in xt_transp_instrs:
                                    tile.add_dep_helper(
                                        ti.ins, xi.ins, sync=False,
                                        reason="pe-order-ht-after-xt",
                                    )
                        ci = nc.vector.tensor_copy(
                            hT[:, :, i0 * 128 : i0 * 128 + 256].rearrange(
                                "p k (t f) -> p t k f", t=2
                            ),
                            ps[:].rearrange("p (t k f) -> p t k f", t=2, k=2),
                        )
                        for xi in xt_evac_instrs:
                            tile.add_dep_helper(
                                ci.ins, xi.ins, sync=False,
                                reason="dve-order-ht-after-xt",
                            )

                # hW = h @ [W1|W2]^T (+ bg on the W2 half)
                for m in range(NT):
                    ps = psf.tile([128, 512], F32, tag="big")
                    nc.tensor.matmul(
                        ps[:],
                        hT[:, 0, ts(m, 128)],
                        w12t[:, 0, :],
                        start=True,
                        stop=False,
                    )
                    nc.tensor.matmul(
                        ps[:],
                        hT[:, 1, ts(m, 128)],
                        w12t[:, 1, :],
                        start=False,
                        stop=False,
                    )
                    nc.tensor.matmul(
                        ps[:, 256:512],
                        ones_row[:],
                        bg_row[:],
                        start=False,
                        stop=True,
                    )
                    # W1 half -> fp8 (alternate DVE/ScalarE), W2 half -> bf16
                    if m % 2 == 0:
                        nc.vector.tensor_copy(hw1[:, m, 0:256], ps[:, 0:256])
                    else:
                        si = nc.scalar.copy(hw1[:, m, 0:256], ps[:, 0:256])
                        # keep ScalarE copies behind phase-1 sqrts in the queue
                        for sq in sqrt_instrs:
                            tile.add_dep_helper(
                                si.ins, sq.ins, sync=False,
                                reason="sc-order-copy-after-sqrt",
                            )
                    nc.vector.tensor_copy(hw2[:, m, :], ps[:, 256:512])

                nc.vector.memset(hw1[:, :, 256:257], 1.0)
                # P = exp(w * dist), full rows, straight to fp8
                for i4 in range(0, NT, 4):
                    ei = nc.scalar.activation(
                        out=p8[:, i4 : i4 + 4, :],
                        in_=d_tiles[b][:, i4 : i4 + 4, :],
                        func=AF.Exp,
                        scale=w_col[:, 0:1],
                    )
                    for si in sqrt_instrs:
                        tile.add_dep_helper(
                            ei.ins, si.ins, sync=False, reason="act-table-order"
                        )

                for i4 in range(0, NT, 4):
                    zs = ztp.tile([128, 4, A], F32, tag="zs")
                    for u in range(4):
                        i = i4 + u
                        pv = psf.tile([128, 512], F32, tag="big")
                        for k2 in range(0, NT, 2):
                            nc.tensor.matmul(
                                pv[:, 0 : A + 1],
                                p8[:, k2 : k2 + 2, ts(i, 128)],
                                hw1[:, k2 : k2 + 2, 0 : A + 1],
                                start=(k2 == 0),
                                stop=(k2 == NT - 2),
                                perf_mode=PM.DoubleRow,
                            )
                        rp_i = smallp.tile([128, 1], F32, tag="rp_i")
                        nc.vector.reciprocal(rp_i[:], pv[:, A : A + 1])
                        nc.vector.scalar_tensor_tensor(
                            out=zs[:, u, :],
                            in0=pv[:, 0:A],
                            scalar=rp_i[:, 0:1],
                            in1=hw2[:, i, :],
                            op0=OP.mult,
                            op1=OP.add,
                        )
                    zo = ztp.tile([128, 4, A], F32, tag="zo")
                    nc.scalar.activation(
                        out=zo[:].rearrange("p a b -> p (a b)"),
                        in_=zs[:].rearrange("p a b -> p (a b)"),
                        func=AF.Tanh,
                    )
                    nc.gpsimd.dma_start(
                        out=out_ext[i4 * 128 : i4 * 128 + 512, b, :].rearrange(
                            "(u p) a -> p u a", p=128
                        ),
                        in_=zo,
                    )

    nc.compile()
    return nc


_CACHED = {}


def _get_graph():
    if "nc" not in _CACHED:
        _CACHED["nc"] = build_graph()
    return _CACHED["nc"]


def _run(inputs, trace=False, **kw):
    nc = _get_graph()
    x = np.asarray(inputs["x"], dtype=np.float32)
    h = np.asarray(inputs["h"], dtype=np.float32)
    w_sim = np.asarray(inputs["w_sim"], dtype=np.float32).reshape(1, 1)
    W_g = np.ascontiguousarray(np.asarray(inputs["W_g"], dtype=np.float32))
    b_g = np.asarray(inputs["b_g"], dtype=np.float32).reshape(1, A)
    in_maps = []
    for c in range(NCORES):
        in_maps.append(
            {
                "x": np.ascontiguousarray(x[:, c * BS : (c + 1) * BS, :]),
                "h": np.ascontiguousarray(h[:, c * BS : (c + 1) * BS, :]),
                "w_sim": w_sim,
                "W_g": W_g,
                "b_g": b_g,
            }
        )
    res = run_bass_kernel_spmd(nc, in_maps, list(range(NCORES)), trace=trace, **kw)
    out = np.concatenate([res.results[c]["out"] for c in range(NCORES)], axis=1)
    return out, res


def kernel(**inputs):
    out, _ = _run(inputs, trace=False)
    return out


if __name__ == "__main__":
    rng = np.random.default_rng(0)
    ins = {
        "x": rng.standard_normal((S, B, E), dtype=np.float32),
        "h": rng.standard_normal((S, B, E), dtype=np.float32),
        "w_sim": np.array([0.03], dtype=np.float32),
        "b_sim": np.array([0.01], dtype=np.float32),
        "W_g": (rng.standard_normal((A, 2 * H)) * 0.05).astype(np.float32),
        "b_g": np.zeros(A, dtype=np.float32),
    }
    out = kernel(**ins)
    print("out", out.shape, out.dtype, np.abs(out).mean())


# revision 14
# speedup vs baseline: 1.8372x; 1.2806x over previous
"""Distance-based attention (nn_Attention_67989332296336) on 8 TRN2 NeuronCores.

Math per batch element b (S=1024, E=H=A=256):
    d2[t,j]  = |x_t|^2 + |x_j|^2 - 2 x_t.x_j
    dist     = sqrt(max(d2,0)+eps)
    scores   = w_sim*dist + b_sim
    A        = softmax_j(scores)
    G        = A @ h
    Z        = tanh([G, h] @ W_g^T + b_g)

Sharding: batch dim B=32 split over 8 cores (4 per core), weights replicated.

Per-core strategy (v3: dist symmetry + fp8 DoubleRow gram/PV):
  - x/h loaded via SWDGE casting DMAs (f32 -> bf16).  x^T built on
    TensorE (bf16 PSUM) and cast to fp8e4 on the DVE evac; h^T for two
    batches via the sync-ring DMA-xbar, the other two on TensorE.
  - gram only for the upper block-triangle as fp8e4 DoubleRow matmuls
    (K=256/instr); "-0.5|x_j|^2" via a bf16 K=8 block-diag aug matmul;
    |x_t|^2+MARGIN is the sqrt bias (scale=-2).  sqrt runs per t-tile
    on the upper cols only; lower dist blocks are TensorE transposes of
    upper ones (dist is symmetric) evacuated by the DVE.
  - exp runs full-width in 4-row chunks, writing P straight to fp8e4;
    PV is DoubleRow fp8 (P pairs x hw1 pairs), with a ones column in
    hw1 giving softmax denominators.  Z = tanh(PV/den + hW2 + bg).
  - hW = h @ [W1|W2]^T per batch; the W1 half is cast to fp8 (DVE and
    ScalarE alternating), the W2 half to bf16 on the DVE; bg added via
    a K=1 aug matmul.
  - ScalarE table discipline: all Sqrt precede all Exp/Tanh.
  - a ~4us dummy-matmul burst trips the PE HAM clock gate under the
    initial DMAs.
"""

import sys

import numpy as np

if "/opt/trn_rl_repo" not in sys.path:
    sys.path.append("/opt/trn_rl_repo")

import concourse.bacc as bacc
import concourse.bass as bass
import concourse.mybir as mybir
import concourse.tile as tile
from concourse.bass import ts
from concourse.bass_utils import run_bass_kernel_spmd
from concourse.masks import make_identity

F32 = mybir.dt.float32
BF16 = mybir.dt.bfloat16
FP8 = mybir.dt.float8e4
AF = mybir.ActivationFunctionType
OP = mybir.AluOpType
PM = mybir.MatmulPerfMode

S = 1024
B = 32
NCORES = 8
BS = B // NCORES  # batches per core
E = 256
H = 256
A = 256
NT = S // 128  # 8 t-tiles
MARGIN = 12.0  # replaces max(d2,0)+eps; absorbs fp8 gram rounding (~cancels in softmax)
HW1P = 272  # hw1 pitch (257 used, padded to %16 for DoubleRow pair strides


def build_graph():
    nc = bacc.Bacc("TRN2", target_bir_lowering=False, debug=False)

    x_ext = nc.declare_dram_parameter("x", [S, BS, E], F32, isOutput=False)
    h_ext = nc.declare_dram_parameter("h", [S, BS, H], F32, isOutput=False)
    w_ext = nc.declare_dram_parameter("w_sim", [1, 1], F32, isOutput=False)
    wg_ext = nc.declare_dram_parameter("W_g", [A, 2 * H], F32, isOutput=False)
    bg_ext = nc.declare_dram_parameter("b_g", [1, A], F32, isOutput=False)
    out_ext = nc.declare_dram_parameter("out", [S, BS, A], F32, isOutput=True)

    with tile.TileContext(nc) as tc:
        with (
            tc.tile_pool(name="consts", bufs=1) as consts,
            tc.tile_pool(name="dist", bufs=BS) as distp,
            tc.tile_pool(name="work", bufs=2) as work,
            tc.tile_pool(name="nat", bufs=4) as natp,
            tc.tile_pool(name="small", bufs=2) as smallp,
            tc.tile_pool(name="zt", bufs=3) as ztp,
            tc.tile_pool(name="ps_bigb", bufs=2, space="PSUM") as psbb,
            tc.tile_pool(name="ps_d2", bufs=2, space="PSUM") as psd,
            tc.tile_pool(name="ps_f32", bufs=2, space="PSUM") as psf,
        ):
            # PE HAM warm-up burst under the initial DMAs
            warm_in = consts.tile([128, 128], BF16)
            nc.vector.memset(warm_in, 1.0)
            warm_ps = psf.tile([128, 512], F32, tag="big")
            for _ in range(44):
                nc.tensor.matmul(
                    warm_ps[:, 0:128], warm_in[:], warm_in[:], start=True, stop=True
                )

            # prefetch inputs; order chosen so phase-1 (x) and the sync-ring
            # h-transposes (h0/h1) are fed as early as possible:
            # x0, h0, x1, h1, x2, x3, h2, h3
            xnat_list = [None] * BS
            hnat_list = [None] * BS
            def load_x(b):
                xnat = natp.tile([128, NT, E], BF16, tag="xnat")
                xnat_list[b] = xnat
                nc.gpsimd.dma_start(
                    out=xnat,
                    in_=x_ext[:, b, :].rearrange("(i p) e -> p i e", p=128),
                )
            def load_h(b):
                hnat = natp.tile([128, NT, H], BF16, tag="hnat")
                hnat_list[b] = hnat
                nc.gpsimd.dma_start(
                    out=hnat,
                    in_=h_ext[:, b, :].rearrange("(i p) e -> p i e", p=128),
                )
            load_x(0); load_h(0); load_x(1); load_h(1)
            load_x(2); load_x(3); load_h(2); load_h(3)

            # ---------------- constants ----------------
            ident = consts.tile([128, 128], F32)
            make_identity(nc, ident)
            identb = consts.tile([128, 128], BF16)
            nc.vector.tensor_copy(identb, ident)
            ones_stage = consts.tile([8, 128], F32)
            nc.vector.memset(ones_stage, 1.0)
            ones_row = consts.tile([1, 128], BF16)
            nc.vector.tensor_copy(ones_row, ones_stage[0:1, :])
            ones8 = consts.tile([8, 128], BF16)
            nc.vector.tensor_copy(ones8, ones_stage)
            zero_stage = consts.tile([8, S], F32)
            nc.vector.memset(zero_stage, 0.0)

            w_col = consts.tile([128, 1], F32)
            nc.sync.dma_start(out=w_col, in_=w_ext[:].partition_broadcast(128))
            bg_stage = consts.tile([1, A], F32)
            nc.sync.dma_start(out=bg_stage, in_=bg_ext[:])
            bg_row = consts.tile([1, A], BF16)
            nc.vector.tensor_copy(bg_row, bg_stage)

            # W_g (A, 2H) -> W12T: 2 k-tiles of (128hd, [A | A]) used as hW rhs
            wnat = consts.tile([128, 2, 2 * H], F32)
            nc.sync.dma_start(
                out=wnat, in_=wg_ext[:].rearrange("(m p) k -> p m k", m=2)
            )
            w12t = consts.tile([128, 2, 2 * H], BF16)
            for k2 in range(2):
                ps = psf.tile([128, 512], F32, tag="big")
                for w in range(2):
                    for m in range(2):
                        nc.tensor.transpose(
                            ps[:, w * 256 + m * 128 : w * 256 + (m + 1) * 128],
                            wnat[:, m, w * 256 + k2 * 128 : w * 256 + (k2 + 1) * 128],
                            ident[:],
                        )
                nc.vector.tensor_copy(w12t[:, k2, :], ps[:])

            # ---------------- phase 1: upper-triangle distances ----------------
            d_tiles = []
            sqrt_instrs = []
            xt_transp_instrs = []  # PE x-transposes (order anchors)
            xt_evac_instrs = []  # DVE x-evacs (order anchors)
            for b in range(BS):
                xT8 = work.tile([128, 2, S], FP8, tag="xT")
                sqmcol = smallp.tile([128, NT], F32, tag="sqm")
                biasp = smallp.tile([128, NT], F32, tag="bias")
                blockdiag = smallp.tile([8, S], BF16, tag="bd")
                nc.vector.tensor_copy(blockdiag, zero_stage)
                d_b = distp.tile([128, NT, S], BF16, tag="D")
                d_tiles.append(d_b)

                xnat = xnat_list[b]

                # x^T: TensorE transpose pairs into bf16 PSUM, DVE evac casts
                # to fp8: psum holds [T0e0|T0e1|T1e0|T1e1]
                for p2 in range(NT // 2):
                    i0, i1 = 2 * p2, 2 * p2 + 1
                    ps = psbb.tile([128, 512], BF16, tag="bigb")
                    for t2, i in enumerate((i0, i1)):
                        for k2 in range(2):
                            ti = nc.tensor.transpose(
                                ps[:, t2 * 256 + k2 * 128 : t2 * 256 + (k2 + 1) * 128],
                                xnat[:, i, ts(k2, 128)],
                                identb[:],
                            )
                            xt_transp_instrs.append(ti)
                    ci = nc.vector.tensor_copy(
                        xT8[:, :, i0 * 128 : i0 * 128 + 256].rearrange(
                            "p k (t f) -> p t k f", t=2
                        ),
                        ps[:].rearrange("p (t k f) -> p t k f", t=2, k=2),
                    )
                    xt_evac_instrs.append(ci)

                # |x_t|^2 per t-tile (DVE); bias = |x_t|^2 + MARGIN
                for i in range(NT):
                    scr = smallp.tile([128, E], F32, tag="scr")
                    nc.vector.scalar_tensor_tensor(
                        out=scr,
                        in0=xnat[:, i, :],
                        scalar=1.0,
                        in1=xnat[:, i, :],
                        op0=OP.mult,
                        op1=OP.mult,
                        accum_out=sqmcol[:, i : i + 1],
                    )
                nc.vector.tensor_scalar_add(out=biasp, in0=sqmcol, scalar1=MARGIN)
                # block-diagonal (8, S) holding -0.5*|x_j|^2
                sqmb = smallp.tile([128, NT], BF16, tag="sqmb")
                nc.vector.tensor_copy(sqmb[:], sqmcol[:])
                sq8 = psbb.tile([8, 128], BF16, tag="bigb")
                nc.tensor.transpose(sq8[:], sqmb[:], identb[:])
                sq8sb = smallp.tile([8, 128], BF16, tag="sq8sb")
                nc.vector.tensor_scalar_mul(sq8sb[:], sq8[:], -0.5)
                bd = blockdiag[:]
                diag_view = bass.AP(
                    tensor=bd.tensor, offset=bd.offset, ap=[[S + 128, NT], [1, 128]]
                )
                nc.gpsimd.dma_start(out=diag_view, in_=sq8sb[:])

                for i in range(NT):
                    lo = 128 * i
                    d2 = psd.tile([128, S], F32, tag="d2")
                    chunks = [(lo, 512), (512, S)] if lo < 512 else [(lo, S)]
                    for c0, c1 in chunks:
                        nc.tensor.matmul(
                            d2[:, c0:c1],
                            xT8[:, :, lo : lo + 128],
                            xT8[:, :, c0:c1],
                            start=True,
                            stop=False,
                            perf_mode=PM.DoubleRow,
                        )
                        nc.tensor.matmul(
                            d2[:, c0:c1],
                            ones8[:],
                            blockdiag[:, c0:c1],
                            start=False,
                            stop=True,
                        )
                    # dist = sqrt(-2*psum + |x_t|^2 + MARGIN) on upper cols
                    si = nc.scalar.activation(
                        out=d_b[:, i, lo:S],
                        in_=d2[:, lo:S],
                        func=AF.Sqrt,
                        bias=biasp[:, i : i + 1],
                        scale=-2.0,
                    )
                    sqrt_instrs.append(si)
                    # lower blocks (k, i), k>i = transposes of row i's upper
                    # blocks; TensorE -> bf16 PSUM -> one DVE evac copy
                    if i < NT - 1:
                        nb = NT - 1 - i
                        tp = psbb.tile([128, 1024], BF16, tag="bigb")
                        for u in range(nb):
                            nc.tensor.transpose(
                                tp[:, u * 128 : (u + 1) * 128],
                                d_b[:, i, (i + 1 + u) * 128 : (i + 2 + u) * 128],
                                identb[:],
                            )
                        nc.vector.tensor_copy(
                            d_b[:, i + 1 :, ts(i, 128)],
                            tp[:, 0 : nb * 128].rearrange(
                                "p (u f) -> p u f", u=nb
                            ),
                        )

            # ---------------- phase 2: softmax + PV + gate ----------------
            for b in range(BS):
                hT = work.tile([128, 2, S], BF16, tag="hT")
                hw1 = work.tile([128, NT, HW1P], FP8, tag="hw1")
                hw2 = work.tile([128, NT, A], BF16, tag="hw2")
                p8 = work.tile([128, NT, S], FP8, tag="P")

                hnat = hnat_list[b]
                # h^T on TensorE like x^T, bf16 evac.  For the late-loaded
                # batches, ordering deps keep these behind all phase-1 x work
                # on the PE/DVE queues so a late h DMA cannot head-block.
                for p2 in range(NT // 2):
                    i0, i1 = 2 * p2, 2 * p2 + 1
                    ps = psbb.tile([128, 512], BF16, tag="bigb")
                    for t2, i in enumerate((i0, i1)):
                        for k2 in range(2):
                            ti = nc.tensor.transpose(
                                ps[
                                    :,
                                    t2 * 256
                                    + k2 * 128 : t2 * 256
                                    + (k2 + 1) * 128,
                                ],
                                hnat[:, i, ts(k2, 128)],
                                identb[:],
                            )
                            if b >= 2:
                                for xi in xt_transp_instrs:
                                    tile.add_dep_helper(
                                        ti.ins, xi.ins, sync=False,
                                        reason="pe-order-ht-after-xt",
                                    )
                    ci = nc.vector.tensor_copy(
                        hT[:, :, i0 * 128 : i0 * 128 + 256].rearrange(
                            "p k (t f) -> p t k f", t=2
                        ),
                        ps[:].rearrange("p (t k f) -> p t k f", t=2, k=2),
                    )
                    if b >= 2:
                        for xi in xt_evac_instrs:
                            tile.add_dep_helper(
                                ci.ins, xi.ins, sync=False,
                                reason="dve-order-ht-after-xt",
                            )

                # hW = h @ [W1|W2]^T (+ bg on the W2 half)
                for m in range(NT):
                    ps = psf.tile([128, 512], F32, tag="big")
                    nc.tensor.matmul(
                        ps[:],
                        hT[:, 0, ts(m, 128)],
                        w12t[:, 0, :],
                        start=True,
                        stop=False,
                    )
                    nc.tensor.matmul(
                        ps[:],
                        hT[:, 1, ts(m, 128)],
                        w12t[:, 1, :],
                        start=False,
                        stop=False,
                    )
                    nc.tensor.matmul(
                        ps[:, 256:512],
                        ones_row[:],
                        bg_row[:],
                        start=False,
                        stop=True,
                    )
                    # W1 half -> fp8 (alternate DVE/ScalarE), W2 half -> bf16
                    if m % 2 == 0:
                        nc.vector.tensor_copy(hw1[:, m, 0:256], ps[:, 0:256])
                    else:
                        si = nc.scalar.copy(hw1[:, m, 0:256], ps[:, 0:256])
                        # keep ScalarE copies behind phase-1 sqrts in the queue
                        for sq in sqrt_instrs:
                            tile.add_dep_helper(
                                si.ins, sq.ins, sync=False,
                                reason="sc-order-copy-after-sqrt",
                            )
                    nc.vector.tensor_copy(hw2[:, m, :], ps[:, 256:512])

                nc.vector.memset(hw1[:, :, 256:257], 1.0)
                # P = exp(w * dist), full rows, straight to fp8
                for i4 in range(0, NT, 4):
                    ei = nc.scalar.activation(
                        out=p8[:, i4 : i4 + 4, :],
                        in_=d_tiles[b][:, i4 : i4 + 4, :],
                        func=AF.Exp,
                        scale=w_col[:, 0:1],
                    )
                    for si in sqrt_instrs:
                        tile.add_dep_helper(
                            ei.ins, si.ins, sync=False, reason="act-table-order"
                        )

                for i4 in range(0, NT, 4):
                    zs = ztp.tile([128, 4, A], F32, tag="zs")
                    for u in range(4):
                        i = i4 + u
                        pv = psf.tile([128, 512], F32, tag="big")
                        for k2 in range(0, NT, 2):
                            nc.tensor.matmul(
                                pv[:, 0 : A + 1],
                                p8[:, k2 : k2 + 2, ts(i, 128)],
                                hw1[:, k2 : k2 + 2, 0 : A + 1],
                                start=(k2 == 0),
                                stop=(k2 == NT - 2),
                                perf_mode=PM.DoubleRow,
                            )
                        rp_i = smallp.tile([128, 1], F32, tag="rp_i")
                        nc.vector.reciprocal(rp_i[:], pv[:, A : A + 1])
                        nc.vector.scalar_tensor_tensor(
                            out=zs[:, u, :],
                            in0=pv[:, 0:A],
                            scalar=rp_i[:, 0:1],
                            in1=hw2[:, i, :],
                            op0=OP.mult,
                            op1=OP.add,
                        )
                    zo = ztp.tile([128, 4, A], F32, tag="zo")
                    nc.scalar.activation(
                        out=zo[:].rearrange("p a b -> p (a b)"),
                        in_=zs[:].rearrange("p a b -> p (a b)"),
                        func=AF.Tanh,
                    )
                    nc.gpsimd.dma_start(
                        out=out_ext[i4 * 128 : i4 * 128 + 512, b, :].rearrange(
                            "(u p) a -> p u a", p=128
                        ),
                        in_=zo,
                    )

    nc.compile()
    return nc


_CACHED = {}


def _get_graph():
    if "nc" not in _CACHED:
        _CACHED["nc"] = build_graph()
    return _CACHED["nc"]


def _run(inputs, trace=False, **kw):
    nc = _get_graph()
    x = np.asarray(inputs["x"], dtype=np.float32)
    h = np.asarray(inputs["h"], dtype=np.float32)
    w_sim = np.asarray(inputs["w_sim"], dtype=np.float32).reshape(1, 1)
    W_g = np.ascontiguousarray(np.asarray(inputs["W_g"], dtype=np.float32))
    b_g = np.asarray(inputs["b_g"], dtype=np.float32).reshape(1, A)
    in_maps = []
    for c in range(NCORES):
        in_maps.append(
            {
                "x": np.ascontiguousarray(x[:, c * BS : (c + 1) * BS, :]),
                "h": np.ascontiguousarray(h[:, c * BS : (c + 1) * BS, :]),
                "w_sim": w_sim,
                "W_g": W_g,
                "b_g": b_g,
            }
        )
    res = run_bass_kernel_spmd(nc, in_maps, list(range(NCORES)), trace=trace, **kw)
    out = np.concatenate([res.results[c]["out"] for c in range(NCORES)], axis=1)
    return out, res


def kernel(**inputs):
    out, _ = _run(inputs, trace=False)
    return out


if __name__ == "__main__":
    rng = np.random.default_rng(0)
    ins = {
        "x": rng.standard_normal((S, B, E), dtype=np.float32),
        "h": rng.standard_normal((S, B, E), dtype=np.float32),
        "w_sim": np.array([0.03], dtype=np.float32),
        "b_sim": np.array([0.01], dtype=np.float32),
        "W_g": (rng.standard_normal((A, 2 * H)) * 0.05).astype(np.float32),
        "b_g": np.zeros(A, dtype=np.float32),
    }
    out = kernel(**ins)
    print("out", out.shape, out.dtype, np.abs(out).mean())


# revision 15
# speedup vs baseline: 1.8464x; 1.0050x over previous
"""Distance-based attention (nn_Attention_67989332296336) on 8 TRN2 NeuronCores.

Math per batch element b (S=1024, E=H=A=256):
    d2[t,j]  = |x_t|^2 + |x_j|^2 - 2 x_t.x_j
    dist     = sqrt(max(d2,0)+eps)
    scores   = w_sim*dist + b_sim
    A        = softmax_j(scores)
    G        = A @ h
    Z        = tanh([G, h] @ W_g^T + b_g)

Sharding: batch dim B=32 split over 8 cores (4 per core), weights replicated.

Per-core strategy (v3: dist symmetry + fp8 DoubleRow gram/PV):
  - x/h loaded via SWDGE casting DMAs (f32 -> bf16).  x^T built on
    TensorE (bf16 PSUM) and cast to fp8e4 on the DVE evac; h^T for two
    batches via the sync-ring DMA-xbar, the other two on TensorE.
  - gram only for the upper block-triangle as fp8e4 DoubleRow matmuls
    (K=256/instr); "-0.5|x_j|^2" via a bf16 K=8 block-diag aug matmul;
    |x_t|^2+MARGIN is the sqrt bias (scale=-2).  sqrt runs per t-tile
    on the upper cols only; lower dist blocks are TensorE transposes of
    upper ones (dist is symmetric) evacuated by the DVE.
  - exp runs full-width in 4-row chunks, writing P straight to fp8e4;
    PV is DoubleRow fp8 (P pairs x hw1 pairs), with a ones column in
    hw1 giving softmax denominators.  Z = tanh(PV/den + hW2 + bg).
  - hW = h @ [W1|W2]^T per batch; the W1 half is cast to fp8 (DVE and
    ScalarE alternating), the W2 half to bf16 on the DVE; bg added via
    a K=1 aug matmul.
  - ScalarE table discipline: all Sqrt precede all Exp/Tanh.
  - a ~4us dummy-matmul burst trips the PE HAM clock gate under the
    initial DMAs.
"""

import sys

import numpy as np

if "/opt/trn_rl_repo" not in sys.path:
    sys.path.append("/opt/trn_rl_repo")

import concourse.bacc as bacc
import concourse.bass as bass
import concourse.mybir as mybir
import concourse.tile as tile
from concourse.bass import ts
from concourse.bass_utils import run_bass_kernel_spmd
from concourse.masks import make_identity

F32 = mybir.dt.float32
BF16 = mybir.dt.bfloat16
FP8 = mybir.dt.float8e4
AF = mybir.ActivationFunctionType
OP = mybir.AluOpType
PM = mybir.MatmulPerfMode

S = 1024
B = 32
NCORES = 8
BS = B // NCORES  # batches per core
E = 256
H = 256
A = 256
NT = S // 128  # 8 t-tiles
MARGIN = 12.0  # replaces max(d2,0)+eps; absorbs fp8 gram rounding (~cancels in softmax)
HW1P = 272  # hw1 pitch (257 used, padded to %16 for DoubleRow pair strides


def build_graph():
    nc = bacc.Bacc("TRN2", target_bir_lowering=False, debug=False)

    x_ext = nc.declare_dram_parameter("x", [BS, S, E], F32, isOutput=False)
    h_ext = nc.declare_dram_parameter("h", [BS, S, H], F32, isOutput=False)
    w_ext = nc.declare_dram_parameter("w_sim", [1, 1], F32, isOutput=False)
    wg_ext = nc.declare_dram_parameter("W_g", [A, 2 * H], F32, isOutput=False)
    bg_ext = nc.declare_dram_parameter("b_g", [1, A], F32, isOutput=False)
    out_ext = nc.declare_dram_parameter("out", [BS, S, A], F32, isOutput=True)

    with tile.TileContext(nc) as tc:
        with (
            tc.tile_pool(name="consts", bufs=1) as consts,
            tc.tile_pool(name="dist", bufs=BS) as distp,
            tc.tile_pool(name="work", bufs=2) as work,
            tc.tile_pool(name="nat", bufs=4) as natp,
            tc.tile_pool(name="small", bufs=2) as smallp,
            tc.tile_pool(name="zt", bufs=3) as ztp,
            tc.tile_pool(name="ps_bigb", bufs=2, space="PSUM") as psbb,
            tc.tile_pool(name="ps_d2", bufs=2, space="PSUM") as psd,
            tc.tile_pool(name="ps_f32", bufs=2, space="PSUM") as psf,
        ):
            # PE HAM warm-up burst under the initial DMAs
            warm_in = consts.tile([128, 128], BF16)
            nc.vector.memset(warm_in, 1.0)
            warm_ps = psf.tile([128, 512], F32, tag="big")
            for _ in range(44):
                nc.tensor.matmul(
                    warm_ps[:, 0:128], warm_in[:], warm_in[:], start=True, stop=True
                )

            # prefetch inputs; order chosen so phase-1 (x) and the sync-ring
            # h-transposes (h0/h1) are fed as early as possible:
            # x0, h0, x1, h1, x2, x3, h2, h3
            xnat_list = [None] * BS
            hnat_list = [None] * BS
            def load_x(b):
                xnat = natp.tile([128, NT, E], BF16, tag="xnat")
                xnat_list[b] = xnat
                nc.gpsimd.dma_start(
                    out=xnat,
                    in_=x_ext[b].rearrange("(p i) e -> p i e", p=128),
                )
            def load_h(b):
                hnat = natp.tile([128, NT, H], BF16, tag="hnat")
                hnat_list[b] = hnat
                nc.gpsimd.dma_start(
                    out=hnat,
                    in_=h_ext[b].rearrange("(p i) e -> p i e", p=128),
                )
            load_x(0); load_h(0); load_x(1); load_h(1)
            load_x(2); load_x(3); load_h(2); load_h(3)

            # ---------------- constants ----------------
            ident = consts.tile([128, 128], F32)
            make_identity(nc, ident)
            identb = consts.tile([128, 128], BF16)
            nc.vector.tensor_copy(identb, ident)
            ones_stage = consts.tile([8, 128], F32)
            nc.vector.memset(ones_stage, 1.0)
            ones_row = consts.tile([1, 128], BF16)
            nc.vector.tensor_copy(ones_row, ones_stage[0:1, :])
            ones8 = consts.tile([8, 128], BF16)
            nc.vector.tensor_copy(ones8, ones_stage)
            zero_stage = consts.tile([8, S], F32)
            nc.vector.memset(zero_stage, 0.0)

            w_col = consts.tile([128, 1], F32)
            nc.sync.dma_start(out=w_col, in_=w_ext[:].partition_broadcast(128))
            bg_stage = consts.tile([1, A], F32)
            nc.sync.dma_start(out=bg_stage, in_=bg_ext[:])
            bg_row = consts.tile([1, A], BF16)
            nc.vector.tensor_copy(bg_row, bg_stage)

            # W_g (A, 2H) -> W12T: 2 k-tiles of (128hd, [A | A]) used as hW rhs
            wnat = consts.tile([128, 2, 2 * H], F32)
            nc.sync.dma_start(
                out=wnat, in_=wg_ext[:].rearrange("(m p) k -> p m k", m=2)
            )
            w12t = consts.tile([128, 2, 2 * H], BF16)
            for k2 in range(2):
                ps = psf.tile([128, 512], F32, tag="big")
                for w in range(2):
                    for m in range(2):
                        nc.tensor.transpose(
                            ps[:, w * 256 + m * 128 : w * 256 + (m + 1) * 128],
                            wnat[:, m, w * 256 + k2 * 128 : w * 256 + (k2 + 1) * 128],
                            ident[:],
                        )
                nc.vector.tensor_copy(w12t[:, k2, :], ps[:])

            # ---------------- phase 1: upper-triangle distances ----------------
            d_tiles = []
            sqrt_instrs = []
            xt_transp_instrs = []  # PE x-transposes (order anchors)
            xt_evac_instrs = []  # DVE x-evacs (order anchors)
            for b in range(BS):
                xT8 = work.tile([128, 2, S], FP8, tag="xT")
                sqmcol = smallp.tile([128, NT], F32, tag="sqm")
                biasp = smallp.tile([128, NT], F32, tag="bias")
                blockdiag = smallp.tile([8, S], BF16, tag="bd")
                nc.vector.tensor_copy(blockdiag, zero_stage)
                d_b = distp.tile([128, NT, S], BF16, tag="D")
                d_tiles.append(d_b)

                xnat = xnat_list[b]

                # x^T: TensorE transpose pairs into bf16 PSUM, DVE evac casts
                # to fp8: psum holds [T0e0|T0e1|T1e0|T1e1]
                for p2 in range(NT // 2):
                    i0, i1 = 2 * p2, 2 * p2 + 1
                    ps = psbb.tile([128, 512], BF16, tag="bigb")
                    for t2, i in enumerate((i0, i1)):
                        for k2 in range(2):
                            ti = nc.tensor.transpose(
                                ps[:, t2 * 256 + k2 * 128 : t2 * 256 + (k2 + 1) * 128],
                                xnat[:, i, ts(k2, 128)],
                                identb[:],
                            )
                            xt_transp_instrs.append(ti)
                    ci = nc.vector.tensor_copy(
                        xT8[:, :, i0 * 128 : i0 * 128 + 256].rearrange(
                            "p k (t f) -> p t k f", t=2
                        ),
                        ps[:].rearrange("p (t k f) -> p t k f", t=2, k=2),
                    )
                    xt_evac_instrs.append(ci)

                # |x_t|^2 per t-tile (DVE); bias = |x_t|^2 + MARGIN
                for i in range(NT):
                    scr = smallp.tile([128, E], F32, tag="scr")
                    nc.vector.scalar_tensor_tensor(
                        out=scr,
                        in0=xnat[:, i, :],
                        scalar=1.0,
                        in1=xnat[:, i, :],
                        op0=OP.mult,
                        op1=OP.mult,
                        accum_out=sqmcol[:, i : i + 1],
                    )
                nc.vector.tensor_scalar_add(out=biasp, in0=sqmcol, scalar1=MARGIN)
                # block-diagonal (8, S) holding -0.5*|x_j|^2
                sqmb = smallp.tile([128, NT], BF16, tag="sqmb")
                nc.vector.tensor_copy(sqmb[:], sqmcol[:])
                sq8 = psbb.tile([8, 128], BF16, tag="bigb")
                nc.tensor.transpose(sq8[:], sqmb[:], identb[:])
                sq8sb = smallp.tile([8, 128], BF16, tag="sq8sb")
                nc.vector.tensor_scalar_mul(sq8sb[:], sq8[:], -0.5)
                bd = blockdiag[:]
                diag_view = bass.AP(
                    tensor=bd.tensor, offset=bd.offset, ap=[[S + 128, NT], [1, 128]]
                )
                nc.gpsimd.dma_start(out=diag_view, in_=sq8sb[:])

                for i in range(NT):
                    lo = 128 * i
                    d2 = psd.tile([128, S], F32, tag="d2")
                    chunks = [(lo, 512), (512, S)] if lo < 512 else [(lo, S)]
                    # HAM keep-warm pulse; junk is overwritten by the
                    # start=True gram mm below
                    nc.tensor.matmul(
                        d2[:, lo : lo + 128], identb[:], identb[:],
                        start=True, stop=True,
                    )
                    for c0, c1 in chunks:
                        nc.tensor.matmul(
                            d2[:, c0:c1],
                            xT8[:, :, lo : lo + 128],
                            xT8[:, :, c0:c1],
                            start=True,
                            stop=False,
                            perf_mode=PM.DoubleRow,
                        )
                        nc.tensor.matmul(
                            d2[:, c0:c1],
                            ones8[:],
                            blockdiag[:, c0:c1],
                            start=False,
                            stop=True,
                        )
                    # dist = sqrt(-2*psum + |x_t|^2 + MARGIN) on upper cols
                    si = nc.scalar.activation(
                        out=d_b[:, i, lo:S],
                        in_=d2[:, lo:S],
                        func=AF.Sqrt,
                        bias=biasp[:, i : i + 1],
                        scale=-2.0,
                    )
                    sqrt_instrs.append(si)
                    # lower blocks (k, i), k>i = transposes of row i's upper
                    # blocks; TensorE -> bf16 PSUM -> one DVE evac copy
                    if i < NT - 1:
                        nb = NT - 1 - i
                        tp = psbb.tile([128, 1024], BF16, tag="bigb")
                        for u in range(nb):
                            nc.tensor.transpose(
                                tp[:, u * 128 : (u + 1) * 128],
                                d_b[:, i, (i + 1 + u) * 128 : (i + 2 + u) * 128],
                                identb[:],
                            )
                        nc.vector.tensor_copy(
                            d_b[:, i + 1 :, ts(i, 128)],
                            tp[:, 0 : nb * 128].rearrange(
                                "p (u f) -> p u f", u=nb
                            ),
                        )

            # ---------------- phase 2: softmax + PV + gate ----------------
            for b in range(BS):
                hT = work.tile([128, 2, S], BF16, tag="hT")
                hw1 = work.tile([128, NT, HW1P], FP8, tag="hw1")
                hw2 = work.tile([128, NT, A], BF16, tag="hw2")
                p8 = work.tile([128, NT, S], FP8, tag="P")

                hnat = hnat_list[b]
                # h^T on TensorE like x^T, bf16 evac.  For the late-loaded
                # batches, ordering deps keep these behind all phase-1 x work
                # on the PE/DVE queues so a late h DMA cannot head-block.
                for p2 in range(NT // 2):
                    i0, i1 = 2 * p2, 2 * p2 + 1
                    ps = psbb.tile([128, 512], BF16, tag="bigb")
                    for t2, i in enumerate((i0, i1)):
                        for k2 in range(2):
                            ti = nc.tensor.transpose(
                                ps[
                                    :,
                                    t2 * 256
                                    + k2 * 128 : t2 * 256
                                    + (k2 + 1) * 128,
                                ],
                                hnat[:, i, ts(k2, 128)],
                                identb[:],
                            )
                            if b >= 2:
                                for xi in xt_transp_instrs:
                                    tile.add_dep_helper(
                                        ti.ins, xi.ins, sync=False,
                                        reason="pe-order-ht-after-xt",
                                    )
                    ci = nc.vector.tensor_copy(
                        hT[:, :, i0 * 128 : i0 * 128 + 256].rearrange(
                            "p k (t f) -> p t k f", t=2
                        ),
                        ps[:].rearrange("p (t k f) -> p t k f", t=2, k=2),
                    )
                    if b >= 2:
                        for xi in xt_evac_instrs:
                            tile.add_dep_helper(
                                ci.ins, xi.ins, sync=False,
                                reason="dve-order-ht-after-xt",
                            )

                # hW = h @ [W1|W2]^T (+ bg on the W2 half)
                for m in range(NT):
                    ps = psf.tile([128, 512], F32, tag="big")
                    nc.tensor.matmul(
                        ps[:, 0:128], identb[:], identb[:], start=True, stop=True
                    )
                    nc.tensor.matmul(
                        ps[:],
                        hT[:, 0, ts(m, 128)],
                        w12t[:, 0, :],
                        start=True,
                        stop=False,
                    )
                    nc.tensor.matmul(
                        ps[:],
                        hT[:, 1, ts(m, 128)],
                        w12t[:, 1, :],
                        start=False,
                        stop=False,
                    )
                    nc.tensor.matmul(
                        ps[:, 256:512],
                        ones_row[:],
                        bg_row[:],
                        start=False,
                        stop=True,
                    )
                    # W1 half -> fp8 (alternate DVE/ScalarE), W2 half -> bf16
                    if m % 2 == 0:
                        nc.vector.tensor_copy(hw1[:, m, 0:256], ps[:, 0:256])
                    else:
                        si = nc.scalar.copy(hw1[:, m, 0:256], ps[:, 0:256])
                        # keep ScalarE copies behind phase-1 sqrts in the queue
                        for sq in sqrt_instrs:
                            tile.add_dep_helper(
                                si.ins, sq.ins, sync=False,
                                reason="sc-order-copy-after-sqrt",
                            )
                    nc.vector.tensor_copy(hw2[:, m, :], ps[:, 256:512])

                nc.vector.memset(hw1[:, :, 256:257], 1.0)
                # P = exp(w * dist), full rows, straight to fp8
                for i4 in range(0, NT, 4):
                    ei = nc.scalar.activation(
                        out=p8[:, i4 : i4 + 4, :],
                        in_=d_tiles[b][:, i4 : i4 + 4, :],
                        func=AF.Exp,
                        scale=w_col[:, 0:1],
                    )
                    for si in sqrt_instrs:
                        tile.add_dep_helper(
                            ei.ins, si.ins, sync=False, reason="act-table-order"
                        )

                for i4 in range(0, NT, 4):
                    zs = ztp.tile([128, 4, A], F32, tag="zs")
                    for u in range(4):
                        i = i4 + u
                        pv = psf.tile([128, 512], F32, tag="big")
                        nc.tensor.matmul(
                            pv[:, 0:128], identb[:], identb[:],
                            start=True, stop=True,
                        )
                        for k2 in range(0, NT, 2):
                            nc.tensor.matmul(
                                pv[:, 0 : A + 1],
                                p8[:, k2 : k2 + 2, ts(i, 128)],
                                hw1[:, k2 : k2 + 2, 0 : A + 1],
                                start=(k2 == 0),
                                stop=(k2 == NT - 2),
                                perf_mode=PM.DoubleRow,
                            )
                        rp_i = smallp.tile([128, 1], F32, tag="rp_i")
                        nc.vector.reciprocal(rp_i[:], pv[:, A : A + 1])
                        nc.vector.scalar_tensor_tensor(
                            out=zs[:, u, :],
                            in0=pv[:, 0:A],
                            scalar=rp_i[:, 0:1],
                            in1=hw2[:, i, :],
                            op0=OP.mult,
                            op1=OP.add,
                        )
                    zo = ztp.tile([128, 4, A], F32, tag="zo")
                    nc.scalar.activation(
                        out=zo[:].rearrange("p a b -> p (a b)"),
                        in_=zs[:].rearrange("p a b -> p (a b)"),
                        func=AF.Tanh,
                    )
                    nc.gpsimd.dma_start(
                        out=out_ext[b].rearrange("(p i) a -> p i a", p=128)[
                            :, i4 : i4 + 4, :
                        ],
                        in_=zo,
                    )

    nc.compile()
    return nc


_CACHED = {}


def _get_graph():
    if "nc" not in _CACHED:
        _CACHED["nc"] = build_graph()
    return _CACHED["nc"]


def _run(inputs, trace=False, **kw):
    nc = _get_graph()
    x = np.asarray(inputs["x"], dtype=np.float32)
    h = np.asarray(inputs["h"], dtype=np.float32)
    w_sim = np.asarray(inputs["w_sim"], dtype=np.float32).reshape(1, 1)
    W_g = np.ascontiguousarray(np.asarray(inputs["W_g"], dtype=np.float32))
    b_g = np.asarray(inputs["b_g"], dtype=np.float32).reshape(1, A)
    in_maps = []
    for c in range(NCORES):
        in_maps.append(
            {
                "x": np.ascontiguousarray(
                    x[:, c * BS : (c + 1) * BS, :].transpose(1, 0, 2)
                ),
                "h": np.ascontiguousarray(
                    h[:, c * BS : (c + 1) * BS, :].transpose(1, 0, 2)
                ),
                "w_sim": w_sim,
                "W_g": W_g,
                "b_g": b_g,
            }
        )
    res = run_bass_kernel_spmd(nc, in_maps, list(range(NCORES)), trace=trace, **kw)
    out = np.concatenate(
        [res.results[c]["out"].transpose(1, 0, 2) for c in range(NCORES)], axis=1
    )
    return out, res


def kernel(**inputs):
    out, _ = _run(inputs, trace=False)
    return out


if __name__ == "__main__":
    rng = np.random.default_rng(0)
    ins = {
        "x": rng.standard_normal((S, B, E), dtype=np.float32),
        "h": rng.standard_normal((S, B, E), dtype=np.float32),
        "w_sim": np.array([0.03], dtype=np.float32),
        "b_sim": np.array([0.01], dtype=np.float32),
        "W_g": (rng.standard_normal((A, 2 * H)) * 0.05).astype(np.float32),
        "b_g": np.zeros(A, dtype=np.float32),
    }
    out = kernel(**ins)
    print("out", out.shape, out.dtype, np.abs(out).mean())
